# revision 5
# baseline (speedup 1.0000x reference)
"""Trainium2 Bass kernel for banded (sliding-window) attention.

Problem: B=8, S=4096, D=1024, window 257 (keys [i-128, i+128]).
Sharding: data-parallel over batch -- 8 batch elements -> 8 NeuronCores.

Per-core program (one batch element, fully on-chip streaming over 16
sequence blocks of 256):
  - PE-transpose x block -> xT [d_in, seq]  (matmul contracts along the
    partition axis, so x must be d-major on chip)
  - qT/kT projections: lhsT = W (resident in SBUF), rhs = xT, fp32r
    matmuls (1 cycle/row); v projection streams Wv from HBM
  - scores for a 384-wide key band (the reference's 768-wide band is
    mostly masked; only 3 of 6 128-strips can ever be valid)
  - additive band mask + exp (fused *1/32 scale + row-sum) on ScalarE
  - PE-transpose of the probabilities, prob @ V, 1/rowsum folded into
    the PSUM->SBUF drain.
"""

import os
import sys

for _p in ("/opt/trn_rl_repo", "/root/.axon_site/_ro/trn_rl_repo"):
    if os.path.isdir(_p) and _p not in sys.path:
        sys.path.insert(0, _p)

import numpy as np

import concourse.bass as bass
import concourse.tile as tile
from concourse import bacc, mybir

F32 = mybir.dt.float32
F32R = mybir.dt.float32r

B, S, D = 8, 4096, 1024
BL = 256          # sequence block
P = 128           # partitions
NK = D // P       # 8 d_in tiles
NM = D // P       # 8 d_out tiles
WIN = 384         # computed score band per 128-query chunk
SCALE = 1.0 / float(np.sqrt(D))
NEG = -1.0e30


def build_nc(seq_len=S):
    nb = seq_len // BL
    nc = bacc.Bacc("TRN2", target_bir_lowering=False, debug=False,
                   num_devices=8)

    x_d = nc.dram_tensor("x", [seq_len, D], F32R, kind="ExternalInput")
    wq_d = nc.dram_tensor("Wq", [D, D], F32R, kind="ExternalInput")
    wk_d = nc.dram_tensor("Wk", [D, D], F32R, kind="ExternalInput")
    wv_d = nc.dram_tensor("Wv", [D, D], F32R, kind="ExternalInput")
    bq_d = nc.dram_tensor("bq2", [P, NM], F32, kind="ExternalInput")
    bk_d = nc.dram_tensor("bk2", [P, NM], F32, kind="ExternalInput")
    bv_d = nc.dram_tensor("bv", [D], F32, kind="ExternalInput")
    mask_d = nc.dram_tensor("mask", [P, WIN], F32, kind="ExternalInput")
    ident_d = nc.dram_tensor("ident", [P, P], F32R, kind="ExternalInput")
    out_d = nc.dram_tensor("out", [seq_len, D], F32, kind="ExternalOutput")

    with tile.TileContext(nc) as tc:
        from contextlib import ExitStack
        with ExitStack() as ctx:
            consts = ctx.enter_context(tc.tile_pool(name="consts", bufs=1))
            xnat_p = ctx.enter_context(tc.tile_pool(name="xnat", bufs=2))
            xt_p = ctx.enter_context(tc.tile_pool(name="xt", bufs=1))
            qt_p = ctx.enter_context(tc.tile_pool(name="qt", bufs=2))
            kt_p = ctx.enter_context(tc.tile_pool(name="kt", bufs=2))
            v_p = ctx.enter_context(tc.tile_pool(name="v", bufs=3))
            wv_p = ctx.enter_context(tc.tile_pool(name="wv", bufs=3))
            es_p = ctx.enter_context(tc.tile_pool(name="es", bufs=2))
            est_p = ctx.enter_context(tc.tile_pool(name="est", bufs=2))
            srp_p = ctx.enter_context(tc.tile_pool(name="srp", bufs=2))
            out_p = ctx.enter_context(tc.tile_pool(name="outp", bufs=2))
            ppsum = ctx.enter_context(
                tc.tile_pool(name="ppsum", bufs=2, space="PSUM"))
            spsum = ctx.enter_context(
                tc.tile_pool(name="spsum", bufs=3, space="PSUM"))
            tpsum = ctx.enter_context(
                tc.tile_pool(name="tpsum", bufs=3, space="PSUM"))

            # ---- one-time constants ----
            wq_sb = consts.tile([P, NK, D], F32R)
            nc.sync.dma_start(
                out=wq_sb, in_=wq_d.ap().rearrange("(k p) m -> p k m", p=P))
            wk_sb = consts.tile([P, NK, D], F32R)
            nc.sync.dma_start(
                out=wk_sb, in_=wk_d.ap().rearrange("(k p) m -> p k m", p=P))
            bq_sb = consts.tile([P, NM], F32)
            nc.sync.dma_start(out=bq_sb, in_=bq_d.ap())
            bk_sb = consts.tile([P, NM], F32)
            nc.sync.dma_start(out=bk_sb, in_=bk_d.ap())
            bv_sb = consts.tile([P, D], F32)
            bv_bcast = bass.AP(tensor=bv_d, offset=0, ap=[[0, P], [1, D]])
            nc.gpsimd.dma_start(out=bv_sb, in_=bv_bcast)
            mask_sb = consts.tile([P, WIN], F32)
            nc.sync.dma_start(out=mask_sb, in_=mask_d.ap())
            ident = consts.tile([P, P], F32R)
            nc.sync.dma_start(out=ident, in_=ident_d.ap())

            qt_tiles = [None] * nb
            kt_tiles = [None] * nb
            v_tiles = [None] * nb

            def project(b):
                x_nat = xnat_p.tile([P, 2, D], F32R, tag="xnat",
                                    name=f"xnat{b}")
                nc.sync.dma_start(
                    out=x_nat,
                    in_=x_d.ap()[b * BL:(b + 1) * BL, :]
                    .rearrange("(t p) d -> p t d", p=P))
                xT = xt_p.tile([P, NK, BL], F32R, tag="xT", name=f"xT{b}")
                for st in range(2):
                    for k in range(NK):
                        pt = tpsum.tile([P, P], F32R, tag="tp",
                                        name=f"tp{b}_{st}_{k}")
                        nc.tensor.transpose(
                            pt, x_nat[:, st, k * P:(k + 1) * P], ident)
                        nc.vector.tensor_copy(
                            xT[:, k, st * P:(st + 1) * P], pt)
                # qT / kT projections
                qt = qt_p.tile([P, NM, BL], F32R, tag="qt", name=f"qt{b}")
                kt = kt_p.tile([P, NM, 2 * BL], F32R, tag="kt", name=f"kt{b}")
                for (w_sb, b_sb, dst, c0) in (
                        (wq_sb, bq_sb, qt, 0), (wk_sb, bk_sb, kt, P)):
                    for m in range(NM):
                        ps = ppsum.tile([P, BL], F32, tag="proj",
                                        name=f"pp{b}_{c0}_{m}")
                        for k in range(NK):
                            nc.tensor.matmul(
                                ps, w_sb[:, k, m * P:(m + 1) * P],
                                xT[:, k, :],
                                start=(k == 0), stop=(k == NK - 1))
                        nc.vector.tensor_scalar_add(
                            dst[:, m, c0:c0 + BL], ps, b_sb[:, m:m + 1])
                qt_tiles[b] = qt
                kt_tiles[b] = kt
                # band halos: ext layout [0:128)=prev tail, [128:384)=own,
                # [384:512)=next head
                if b > 0:
                    nc.vector.tensor_copy(
                        kt[:, :, 0:P], kt_tiles[b - 1][:, :, BL:BL + P])
                    nc.vector.tensor_copy(
                        kt_tiles[b - 1][:, :, BL + P:2 * BL], kt[:, :, P:2 * P])
                # v projection, Wv streamed
                vt = v_p.tile([P, 2, D], F32R, tag="v", name=f"v{b}")
                for n in range(2):
                    psA = ppsum.tile([P, 512], F32, tag="proj",
                                     name=f"pva{b}_{n}")
                    psB = ppsum.tile([P, 512], F32, tag="proj",
                                     name=f"pvb{b}_{n}")
                    for k in range(NK):
                        wv_sb = wv_p.tile([P, 512], F32R, tag="wv",
                                          name=f"wv{b}_{n}_{k}")
                        nc.sync.dma_start(
                            out=wv_sb,
                            in_=wv_d.ap()[k * P:(k + 1) * P,
                                          n * 512:(n + 1) * 512])
                        nc.tensor.matmul(psA, xT[:, k, 0:P], wv_sb,
                                         start=(k == 0), stop=(k == NK - 1))
                        nc.tensor.matmul(psB, xT[:, k, P:2 * P], wv_sb,
                                         start=(k == 0), stop=(k == NK - 1))
                    nc.vector.tensor_add(
                        vt[:, 0, n * 512:(n + 1) * 512], psA,
                        bv_sb[:, n * 512:(n + 1) * 512])
                    nc.vector.tensor_add(
                        vt[:, 1, n * 512:(n + 1) * 512], psB,
                        bv_sb[:, n * 512:(n + 1) * 512])
                v_tiles[b] = vt

            def attend(b):
                outp = out_p.tile([P, 2, D], F32, tag="out", name=f"out{b}")
                est = est_p.tile([P, 6, P], F32R, tag="est", name=f"est{b}")
                srp = srp_p.tile([P, 4], F32, tag="srp", name=f"srp{b}")
                for qc in range(2):
                    if b == 0 and qc == 0:
                        wstart, wlen, m0 = P, 2 * P, P
                    elif b == nb - 1 and qc == 1:
                        wstart, wlen, m0 = P, 2 * P, 0
                    else:
                        wstart, wlen, m0 = P * qc, 3 * P, 0
                    sc = spsum.tile([P, 512], F32, tag="sc",
                                    name=f"sc{b}_{qc}")
                    for k in range(NK):
                        nc.tensor.matmul(
                            sc[:, 0:wlen],
                            qt_tiles[b][:, k, qc * P:(qc + 1) * P],
                            kt_tiles[b][:, k, wstart:wstart + wlen],
                            start=(k == 0), stop=(k == NK - 1))
                    nc.vector.tensor_add(
                        sc[:, 0:wlen], sc[:, 0:wlen],
                        mask_sb[:, m0:m0 + wlen])
                    es = es_p.tile([P, WIN], F32R, tag="es",
                                   name=f"es{b}_{qc}")
                    nc.scalar.activation(
                        es[:, 0:wlen], sc[:, 0:wlen],
                        mybir.ActivationFunctionType.Exp,
                        bias=0.0, scale=SCALE,
                        accum_out=srp[:, 2 * qc:2 * qc + 1])
                    nc.vector.reciprocal(
                        srp[:, 2 * qc + 1:2 * qc + 2],
                        srp[:, 2 * qc:2 * qc + 1])
                    nst = wlen // P
                    j0 = wstart // P
                    for i in range(nst):
                        pt = tpsum.tile([P, P], F32R, tag="tp",
                                        name=f"et{b}_{qc}_{i}")
                        nc.tensor.transpose(pt, es[:, i * P:(i + 1) * P],
                                            ident)
                        nc.vector.tensor_copy(est[:, qc * 3 + i, :], pt)
                    for n in range(2):
                        av = spsum.tile([P, 512], F32, tag="sc",
                                        name=f"av{b}_{qc}_{n}")
                        for i in range(nst):
                            t = 2 * b - 1 + j0 + i
                            vt = v_tiles[t // 2]
                            nc.tensor.matmul(
                                av, est[:, qc * 3 + i, :],
                                vt[:, t % 2, n * 512:(n + 1) * 512],
                                start=(i == 0), stop=(i == nst - 1))
                        nc.vector.tensor_scalar_mul(
                            outp[:, qc, n * 512:(n + 1) * 512], av,
                            srp[:, 2 * qc + 1:2 * qc + 2])
                nc.scalar.dma_start(
                    out=out_d.ap()[b * BL:(b + 1) * BL, :]
                    .rearrange("(q p) d -> p q d", p=P),
                    in_=outp)

            project(0)
            for b in range(nb):
                if b + 1 < nb:
                    project(b + 1)
                attend(b)

    nc.compile()
    return nc


def band_mask():
    r = np.arange(P)[:, None]
    c = np.arange(WIN)[None, :]
    valid = (c >= r) & (c <= r + 2 * P)
    return np.where(valid, 0.0, NEG).astype(np.float32)


def host_inputs(x_b, Wq, bq, Wk, bk, Wv, bv):
    return {
        "x": np.ascontiguousarray(x_b, dtype=np.float32),
        "Wq": np.ascontiguousarray(Wq, dtype=np.float32),
        "Wk": np.ascontiguousarray(Wk, dtype=np.float32),
        "Wv": np.ascontiguousarray(Wv, dtype=np.float32),
        "bq2": np.ascontiguousarray(
            np.asarray(bq, dtype=np.float32).reshape(NM, P).T),
        "bk2": np.ascontiguousarray(
            np.asarray(bk, dtype=np.float32).reshape(NM, P).T),
        "bv": np.ascontiguousarray(bv, dtype=np.float32),
        "mask": band_mask(),
        "ident": np.eye(P, dtype=np.float32),
    }


_NC = None


def kernel(x, Wq, bq, Wk, bk, Wv, bv):
    from concourse.bass_utils import run_bass_kernel_spmd
    global _NC
    if _NC is None:
        _NC = build_nc(S)
    x = np.asarray(x, dtype=np.float32)
    in_maps = [host_inputs(x[b], Wq, bq, Wk, bk, Wv, bv) for b in range(B)]
    res = run_bass_kernel_spmd(_NC, in_maps, list(range(B)))
    out = np.stack([res.results[b]["out"] for b in range(B)], axis=0)
    return out.astype(np.float32)


# revision 17
# speedup vs baseline: 1.0097x; 1.0097x over previous
"""Trainium2 Bass kernel for banded (sliding-window) attention.

Problem: B=8, S=4096, D=1024, window 257 (keys [i-128, i+128]).
Sharding: data-parallel over batch -- 8 batch elements -> 8 NeuronCores.

Per-core program (one batch element, fully on-chip streaming over 16
sequence blocks of 256):
  - PE-transpose x block -> xT [d_in, seq]  (matmul contracts along the
    partition axis, so x must be d-major on chip)
  - qT/kT projections: lhsT = W (resident in SBUF), rhs = xT, fp32r
    matmuls (1 cycle/row); v projection streams Wv from HBM
  - scores for a 384-wide key band (the reference's 768-wide band is
    mostly masked; only 3 of 6 128-strips can ever be valid)
  - additive band mask + exp (fused *1/32 scale + row-sum) on ScalarE
  - PE-transpose of the probabilities, prob @ V, 1/rowsum folded into
    the PSUM->SBUF drain.
"""

import os
import sys

for _p in ("/opt/trn_rl_repo", "/root/.axon_site/_ro/trn_rl_repo"):
    if os.path.isdir(_p) and _p not in sys.path:
        sys.path.insert(0, _p)

import numpy as np

import concourse.bass as bass
import concourse.tile as tile
from concourse import bacc, mybir

F32 = mybir.dt.float32
F32R = mybir.dt.float32r

B, S, D = 8, 4096, 1024
BL = 256          # sequence block
P = 128           # partitions
NK = D // P       # 8 d_in tiles
NM = D // P       # 8 d_out tiles
WIN = 384         # computed score band per 128-query chunk
SCALE = 1.0 / float(np.sqrt(D))
NEG = -1.0e30


DEFAULT_CFG = dict(xnat=2, xt=1, qt=2, kt=2, v=3, wv=6, es=2, est=2,
                   srp=2, outp=2, ppsum=2, spsum=3, tpsum=3)


def build_nc(seq_len=S, cfg=None):
    cfg = {**DEFAULT_CFG, **(cfg or {})}
    nb = seq_len // BL
    nc = bacc.Bacc("TRN2", target_bir_lowering=False, debug=False,
                   num_devices=8)

    x_d = nc.dram_tensor("x", [seq_len, D], F32R, kind="ExternalInput")
    wq_d = nc.dram_tensor("Wq", [D, D], F32R, kind="ExternalInput")
    wk_d = nc.dram_tensor("Wk", [D, D], F32R, kind="ExternalInput")
    wv_d = nc.dram_tensor("Wv", [D, D], F32R, kind="ExternalInput")
    bq_d = nc.dram_tensor("bq2", [P, NM], F32, kind="ExternalInput")
    bk_d = nc.dram_tensor("bk2", [P, NM], F32, kind="ExternalInput")
    bv_d = nc.dram_tensor("bv", [D], F32, kind="ExternalInput")
    mask_d = nc.dram_tensor("mask", [P, WIN], F32, kind="ExternalInput")
    ident_d = nc.dram_tensor("ident", [P, P], F32R, kind="ExternalInput")
    out_d = nc.dram_tensor("out", [seq_len, D], F32, kind="ExternalOutput")

    with tile.TileContext(nc) as tc:
        from contextlib import ExitStack
        with ExitStack() as ctx:
            def pool(name, space="SBUF"):
                return ctx.enter_context(
                    tc.tile_pool(name=name, bufs=cfg.get(name, 2),
                                 space=space))

            consts = ctx.enter_context(tc.tile_pool(name="consts", bufs=1))
            xnat_p = pool("xnat")
            xt_p = pool("xt")
            qt_p = pool("qt")
            kt_p = pool("kt")
            v_p = pool("v")
            wv_p = pool("wv")
            es_p = pool("es")
            est_p = pool("est")
            srp_p = pool("srp")
            out_p = pool("outp")
            ppsum = pool("ppsum", space="PSUM")
            spsum = pool("spsum", space="PSUM")
            tpsum = pool("tpsum", space="PSUM")

            # ---- one-time constants (small ones first so the identity /
            # mask don't queue behind 8MB of weights) ----
            ident = consts.tile([P, P], F32R)
            nc.sync.dma_start(out=ident, in_=ident_d.ap())
            mask_sb = consts.tile([P, WIN], F32)
            nc.sync.dma_start(out=mask_sb, in_=mask_d.ap())
            bq_sb = consts.tile([P, NM], F32)
            nc.sync.dma_start(out=bq_sb, in_=bq_d.ap())
            bk_sb = consts.tile([P, NM], F32)
            nc.sync.dma_start(out=bk_sb, in_=bk_d.ap())
            bv_sb = consts.tile([P, D], F32)
            bv_bcast = bass.AP(tensor=bv_d, offset=0, ap=[[0, P], [1, D]])
            nc.gpsimd.dma_start(out=bv_sb, in_=bv_bcast)
            wq_sb = consts.tile([P, NK, D], F32R)
            wk_sb = consts.tile([P, NK, D], F32R)

            def load_weights():
                for k in range(NK):
                    nc.sync.dma_start(out=wq_sb[:, k, :],
                                      in_=wq_d.ap()[k * P:(k + 1) * P, :])
                    nc.scalar.dma_start(out=wk_sb[:, k, :],
                                        in_=wk_d.ap()[k * P:(k + 1) * P, :])

            qt_tiles = [None] * nb
            kt_tiles = [None] * nb
            v_tiles = [None] * nb

            def load_x(b):
                x_nat = xnat_p.tile([P, 2, D], F32R, tag="xnat",
                                    name=f"xnat{b}")
                nc.sync.dma_start(
                    out=x_nat,
                    in_=x_d.ap()[b * BL:(b + 1) * BL, :]
                    .rearrange("(t p) d -> p t d", p=P))
                xT = xt_p.tile([P, NK, BL], F32R, tag="xT", name=f"xT{b}")
                for st in range(2):
                    for k in range(NK):
                        pt = tpsum.tile([P, P], F32R, tag="tp",
                                        name=f"tp{b}_{st}_{k}")
                        nc.tensor.transpose(
                            pt, x_nat[:, st, k * P:(k + 1) * P], ident)
                        nc.vector.tensor_copy(
                            xT[:, k, st * P:(st + 1) * P], pt)
                return xT

            def _proj_v(b, xT):
                vt = v_p.tile([P, 2, D], F32R, tag="v", name=f"v{b}")
                for n in range(2):
                    psA = ppsum.tile([P, 512], F32, tag="proj",
                                     name=f"pva{b}_{n}")
                    psB = ppsum.tile([P, 512], F32, tag="proj",
                                     name=f"pvb{b}_{n}")
                    for k in range(NK):
                        wv_sb = wv_p.tile([P, 512], F32R, tag="wv",
                                          name=f"wv{b}_{n}_{k}")
                        nc.sync.dma_start(
                            out=wv_sb,
                            in_=wv_d.ap()[k * P:(k + 1) * P,
                                          n * 512:(n + 1) * 512])
                        nc.tensor.matmul(psA, xT[:, k, 0:P], wv_sb,
                                         start=(k == 0), stop=(k == NK - 1))
                        nc.tensor.matmul(psB, xT[:, k, P:2 * P], wv_sb,
                                         start=(k == 0), stop=(k == NK - 1))
                    nc.vector.tensor_add(
                        vt[:, 0, n * 512:(n + 1) * 512], psA,
                        bv_sb[:, n * 512:(n + 1) * 512])
                    nc.vector.tensor_add(
                        vt[:, 1, n * 512:(n + 1) * 512], psB,
                        bv_sb[:, n * 512:(n + 1) * 512])
                v_tiles[b] = vt

            def _proj_qk(b, xT):
                qt = qt_p.tile([P, NM, BL], F32R, tag="qt", name=f"qt{b}")
                kt = kt_p.tile([P, NM, 2 * BL], F32R, tag="kt", name=f"kt{b}")
                for (w_sb, b_sb, dst, c0) in (
                        (wq_sb, bq_sb, qt, 0), (wk_sb, bk_sb, kt, P)):
                    for m in range(NM):
                        ps = ppsum.tile([P, BL], F32, tag="proj",
                                        name=f"pp{b}_{c0}_{m}")
                        for k in range(NK):
                            nc.tensor.matmul(
                                ps, w_sb[:, k, m * P:(m + 1) * P],
                                xT[:, k, :],
                                start=(k == 0), stop=(k == NK - 1))
                        nc.vector.tensor_scalar_add(
                            dst[:, m, c0:c0 + BL], ps, b_sb[:, m:m + 1])
                qt_tiles[b] = qt
                kt_tiles[b] = kt
                # band halos: ext layout [0:128)=prev tail, [128:384)=own,
                # [384:512)=next head
                if b > 0:
                    nc.vector.tensor_copy(
                        kt[:, :, 0:P], kt_tiles[b - 1][:, :, BL:BL + P])
                    nc.vector.tensor_copy(
                        kt_tiles[b - 1][:, :, BL + P:2 * BL], kt[:, :, P:2 * P])

            def attend(b):
                outp = out_p.tile([P, 2, D], F32, tag="out", name=f"out{b}")
                est = est_p.tile([P, 6, P], F32R, tag="est", name=f"est{b}")
                srp = srp_p.tile([P, 4], F32, tag="srp", name=f"srp{b}")
                for qc in range(2):
                    if b == 0 and qc == 0:
                        wstart, wlen, m0 = P, 2 * P, P
                    elif b == nb - 1 and qc == 1:
                        wstart, wlen, m0 = P, 2 * P, 0
                    else:
                        wstart, wlen, m0 = P * qc, 3 * P, 0
                    sc = spsum.tile([P, 512], F32, tag="sc",
                                    name=f"sc{b}_{qc}")
                    for k in range(NK):
                        nc.tensor.matmul(
                            sc[:, 0:wlen],
                            qt_tiles[b][:, k, qc * P:(qc + 1) * P],
                            kt_tiles[b][:, k, wstart:wstart + wlen],
                            start=(k == 0), stop=(k == NK - 1))
                    nc.vector.tensor_add(
                        sc[:, 0:wlen], sc[:, 0:wlen],
                        mask_sb[:, m0:m0 + wlen])
                    es = es_p.tile([P, WIN], F32R, tag="es",
                                   name=f"es{b}_{qc}")
                    nc.scalar.activation(
                        es[:, 0:wlen], sc[:, 0:wlen],
                        mybir.ActivationFunctionType.Exp,
                        bias=0.0, scale=SCALE,
                        accum_out=srp[:, 2 * qc:2 * qc + 1])
                    nc.vector.reciprocal(
                        srp[:, 2 * qc + 1:2 * qc + 2],
                        srp[:, 2 * qc:2 * qc + 1])
                    nst = wlen // P
                    j0 = wstart // P
                    for i in range(nst):
                        pt = tpsum.tile([P, P], F32R, tag="tp",
                                        name=f"et{b}_{qc}_{i}")
                        nc.tensor.transpose(pt, es[:, i * P:(i + 1) * P],
                                            ident)
                        nc.vector.tensor_copy(est[:, qc * 3 + i, :], pt)
                    for n in range(2):
                        av = spsum.tile([P, 512], F32, tag="sc",
                                        name=f"av{b}_{qc}_{n}")
                        for i in range(nst):
                            t = 2 * b - 1 + j0 + i
                            vt = v_tiles[t // 2]
                            nc.tensor.matmul(
                                av, est[:, qc * 3 + i, :],
                                vt[:, t % 2, n * 512:(n + 1) * 512],
                                start=(i == 0), stop=(i == nst - 1))
                        nc.vector.tensor_scalar_mul(
                            outp[:, qc, n * 512:(n + 1) * 512], av,
                            srp[:, 2 * qc + 1:2 * qc + 2])
                nc.scalar.dma_start(
                    out=out_d.ap()[b * BL:(b + 1) * BL, :]
                    .rearrange("(q p) d -> p q d", p=P),
                    in_=outp)

            # prologue: block-0 x-load + v-projection run off the small
            # streamed DMAs so PE starts immediately; the 8MB Wq/Wk loads
            # are only queued afterwards
            xT0 = load_x(0)
            _proj_v(0, xT0)
            load_weights()
            _proj_qk(0, xT0)
            for b in range(nb):
                if b + 1 < nb:
                    xT = load_x(b + 1)
                    _proj_qk(b + 1, xT)
                    _proj_v(b + 1, xT)
                attend(b)

    nc.compile()
    return nc


def band_mask():
    r = np.arange(P)[:, None]
    c = np.arange(WIN)[None, :]
    valid = (c >= r) & (c <= r + 2 * P)
    return np.where(valid, 0.0, NEG).astype(np.float32)


def host_inputs(x_b, Wq, bq, Wk, bk, Wv, bv):
    return {
        "x": np.ascontiguousarray(x_b, dtype=np.float32),
        "Wq": np.ascontiguousarray(Wq, dtype=np.float32),
        "Wk": np.ascontiguousarray(Wk, dtype=np.float32),
        "Wv": np.ascontiguousarray(Wv, dtype=np.float32),
        "bq2": np.ascontiguousarray(
            np.asarray(bq, dtype=np.float32).reshape(NM, P).T),
        "bk2": np.ascontiguousarray(
            np.asarray(bk, dtype=np.float32).reshape(NM, P).T),
        "bv": np.ascontiguousarray(bv, dtype=np.float32),
        "mask": band_mask(),
        "ident": np.eye(P, dtype=np.float32),
    }


_NC = None


def kernel(x, Wq, bq, Wk, bk, Wv, bv):
    from concourse.bass_utils import run_bass_kernel_spmd
    global _NC
    if _NC is None:
        _NC = build_nc(S)
    x = np.asarray(x, dtype=np.float32)
    in_maps = [host_inputs(x[b], Wq, bq, Wk, bk, Wv, bv) for b in range(B)]
    res = run_bass_kernel_spmd(_NC, in_maps, list(range(B)))
    out = np.stack([res.results[b]["out"] for b in range(B)], axis=0)
    return out.astype(np.float32)
